# revision 4
# baseline (speedup 1.0000x reference)
"""Cosine attention kernel for Trainium2, sharded over 8 NeuronCores.

Problem: N=4, L=S=2048, H=8, D=64 fp32.
  q = queries / ||queries||_D ; k = keys / ||keys||_D
  qk = einsum('nlhd,nshd->nlsh', q, k); A = softmax(qk / temp, axis=S)
  out = einsum('nlsh,nshd->nlhd', A, values)

Sharding: the 32 (n, h) pairs are split 4-per-core (data + head parallel).
Each core computes 4 independent 2048x2048 attention problems.

Per-core device algorithm (per pair):
  - load Q, K as [128, 16, 64] tiles (L/S on partitions), V as [128, 16, 65]
    with a ones-column appended (row 64 of the second matmul's output then
    accumulates the softmax denominator).
  - row norms: ssq via DVE square+reduce; rsqrt via ACT ln/exp (keeps all
    ACT traffic in one activation-table set with the softmax Exp);
    1/temp folded into K's row scale.
  - normalize Q,K rows (DVE per-partition scalar), PE-transpose to get
    QnT/KnT [64(D), 2048] — matmul operands with D on partitions.
  - scores transposed: P^T[s_tile, l] = KnT_tile^T @ QnT (fp32r matmuls,
    N=512) into PSUM [128, 1024]; ACT Exp PSUM->SBUF.
  - out^T accumulation: psum2[65, 1024] += V_aug[s]^T @ Pexp[s] over 16
    s-tiles (fp32r); row 64 accumulates sum_s exp = softmax denominator.
  - epilogue: PE-transpose [65,128] blocks back to [128,65], DVE reciprocal
    of the denominator column, per-partition scalar multiply, DMA out.
"""

import sys

if "/opt/trn_rl_repo" not in sys.path:
    sys.path.insert(0, "/opt/trn_rl_repo")

import numpy as np

N_CORES = 8
PAIRS = 4          # (n, h) pairs per core
L = 2048           # query length
S = 2048           # key length
D = 64             # head dim
T = S // 128       # 128-row tiles per pair
LC = 2             # L chunks
LCHUNK = L // LC   # 1024

_PROGRAM_CACHE = {}


def _build_program():
    import concourse.tile as tile
    from concourse import bacc, mybir
    from concourse.bass import ds
    from concourse.masks import make_identity

    f32 = mybir.dt.float32
    f32r = mybir.dt.float32r
    AF = mybir.ActivationFunctionType

    nc = bacc.Bacc("TRN2", target_bir_lowering=False, debug=False,
                   num_devices=N_CORES)
    q_hbm = nc.dram_tensor("q", [PAIRS, L, D], f32, kind="ExternalInput")
    k_hbm = nc.dram_tensor("k", [PAIRS, S, D], f32, kind="ExternalInput")
    v_hbm = nc.dram_tensor("v", [PAIRS, S, D], f32, kind="ExternalInput")
    t_hbm = nc.dram_tensor("temp", [1, 1], f32, kind="ExternalInput")
    o_hbm = nc.dram_tensor("o", [PAIRS, L, D], f32, kind="ExternalOutput")

    with tile.TileContext(nc) as tc:
        with (
            tc.tile_pool(name="const", bufs=1) as cpool,
            tc.tile_pool(name="raw", bufs=1) as raw_pool,
            tc.tile_pool(name="io", bufs=2) as io_pool,
            tc.tile_pool(name="work", bufs=2) as work_pool,
            tc.tile_pool(name="small", bufs=4) as small_pool,
            tc.tile_pool(name="pexp", bufs=3) as pexp_pool,
            tc.tile_pool(name="psum1", bufs=2, space="PSUM") as psum1_pool,
            tc.tile_pool(name="psum2", bufs=1, space="PSUM") as psum2_pool,
            tc.tile_pool(name="psmall", bufs=2, space="PSUM") as psmall_pool,
            tc.tile_pool(name="dram", bufs=1, space="DRAM") as dram_pool,
        ):
            identity = cpool.tile([128, 128], f32)
            make_identity(nc, identity[:])

            # HAM warmup: ~32 dense REGULAR matmuls trip the PE clock gate
            # to K=8/8 (2.4 GHz) while the input DMAs stream in. Transpose-
            # mode PE work does not count as HAM activity, so without this
            # the whole prologue runs at 1.2 GHz.
            ps_w = psmall_pool.tile([128, 128], f32, tag="tp", name="ps_w")
            for _ in range(32):
                nc.tensor.matmul(ps_w[:], identity[:], identity[:])
            warm_sink = cpool.tile([1, 1], f32)
            nc.vector.tensor_copy(warm_sink[:], ps_w[0:1, 0:1])
            warm_dram = dram_pool.tile([1, 1], f32)
            nc.sync.dma_start(warm_dram[:], warm_sink[:])

            # 1/temp broadcast to [128, 1] (bounce through DRAM for the
            # partition-broadcast DMA).
            t_sb = cpool.tile([1, 1], f32)
            nc.sync.dma_start(t_sb[:], t_hbm.ap())
            rt_sb = cpool.tile([1, 1], f32)
            nc.vector.reciprocal(rt_sb[:], t_sb[:])
            rt_dram = dram_pool.tile([1, 1], f32)
            nc.sync.dma_start(rt_dram[:], rt_sb[:])
            rt_b = cpool.tile([128, 1], f32)
            nc.sync.dma_start(rt_b[:], rt_dram[:].to_broadcast([128, 1]))

            # ---- Phase 0: load Q/K for all pairs, compute row-norm scales.
            q_raw, k_raw, rq, rk = {}, {}, {}, {}
            for p in range(PAIRS):
                q_raw[p] = raw_pool.tile([128, T, D], f32, tag=f"qraw{p}", name=f"qraw{p}")
                nc.sync.dma_start(
                    q_raw[p][:],
                    q_hbm.ap()[p].rearrange("(t pp) d -> pp t d", pp=128))
                k_raw[p] = raw_pool.tile([128, T, D], f32, tag=f"kraw{p}", name=f"kraw{p}")
                nc.sync.dma_start(
                    k_raw[p][:],
                    k_hbm.ap()[p].rearrange("(t pp) d -> pp t d", pp=128))

            # ssq for all pairs first, then ALL Ln, then ALL Exp — the ACT
            # engine is strict FIFO, so this costs 2 activation-table loads
            # instead of 16 (Ln and Exp live in different table sets).
            ssqs = []
            for p in range(PAIRS):
                for name, srct in (("q", q_raw[p]), ("k", k_raw[p])):
                    sq = work_pool.tile([128, T, D], f32, tag="sq")
                    nc.vector.tensor_mul(sq[:], srct[:], srct[:])
                    ssq = small_pool.tile([128, T], f32, tag=f"ssq_{name}{p}",
                                          name=f"ssq_{name}{p}")
                    nc.vector.tensor_reduce(
                        ssq[:], sq[:], axis=mybir.AxisListType.X,
                        op=mybir.AluOpType.add)
                    ssqs.append((p, name, ssq))
            for p, name, ssq in ssqs:
                nc.scalar.activation(ssq[:], ssq[:], AF.Ln)
            for p, name, ssq in ssqs:
                r = raw_pool.tile([128, T], f32, tag=f"r{name}{p}", name=f"r{name}{p}")
                nc.scalar.activation(r[:], ssq[:], AF.Exp, scale=-0.5)
                (rq if name == "q" else rk)[p] = r
            for p in range(PAIRS):
                # fold 1/temp into K's row scale (idle GpSimd engine)
                nc.gpsimd.tensor_scalar_mul(rk[p][:], rk[p][:], rt_b[:])

            # ---- Per-pair pipeline.
            for p in range(PAIRS):
                # V with ones column appended; converted to f32r for mm2.
                v_stage = io_pool.tile([128, T, D + 1], f32, tag="vstage")
                nc.vector.memset(v_stage[:, :, D:D + 1], 1.0)
                nc.sync.dma_start(
                    v_stage[:, :, 0:D],
                    v_hbm.ap()[p].rearrange("(t pp) d -> pp t d", pp=128))
                v_aug = io_pool.tile([128, T, D + 1], f32r, tag="vaug")
                nc.gpsimd.tensor_copy(v_aug[:], v_stage[:])

                # Normalize rows.
                qn = work_pool.tile([128, T, D], f32, tag="qn")
                kn = work_pool.tile([128, T, D], f32, tag="kn")
                for t in range(T):
                    nc.gpsimd.tensor_scalar_mul(
                        qn[:, t, :], q_raw[p][:, t, :], rq[p][:, t:t + 1])
                    nc.gpsimd.tensor_scalar_mul(
                        kn[:, t, :], k_raw[p][:, t, :], rk[p][:, t:t + 1])

                # Transpose to [64(D), 2048].
                qnT = work_pool.tile([64, L], f32r, tag="qnT")
                knT = work_pool.tile([64, S], f32r, tag="knT")
                for src, dst in ((qn, qnT), (kn, knT)):
                    for g in range(T // 4):
                        tp = psmall_pool.tile([64, 4, 128], f32, tag="tp")
                        for j in range(4):
                            nc.tensor.transpose(
                                tp[:, j, :], src[:, 4 * g + j, :], identity[:])
                        nc.vector.tensor_copy(dst[:, ds(512 * g, 512)], tp[:])

                # Main loop.
                for lc in range(LC):
                    ps2 = psum2_pool.tile([D + 1, LCHUNK], f32, tag="ps2")
                    for st in range(T):
                        ps1 = psum1_pool.tile([128, LCHUNK], f32, tag="ps1")
                        lhs1 = knT[:, ds(st * 128, 128)]
                        for h in range(LCHUNK // 512):
                            nc.tensor.matmul(
                                ps1[:, ds(h * 512, 512)], lhs1,
                                qnT[:, ds(lc * LCHUNK + h * 512, 512)])
                        pexp = pexp_pool.tile([128, LCHUNK], f32r, tag="pexp")
                        nc.scalar.activation(pexp[:], ps1[:], AF.Exp)
                        lhs2 = v_aug[:, st, :]
                        for h in range(LCHUNK // 512):
                            nc.tensor.matmul(
                                ps2[:, ds(h * 512, 512)], lhs2,
                                pexp[:, ds(h * 512, 512)],
                                start=(st == 0), stop=(st == T - 1))

                    # Epilogue for this L chunk.
                    o_sb = work_pool.tile([D + 1, LCHUNK], f32, tag="osb")
                    nc.vector.tensor_copy(o_sb[:], ps2[:])
                    for j in range(LCHUNK // 128):
                        tp = psmall_pool.tile([128, D + 1], f32, tag="tp")
                        nc.tensor.transpose(
                            tp[:], o_sb[:, ds(j * 128, 128)],
                            identity[0:D + 1, 0:D + 1])
                        rcp = small_pool.tile([128, 1], f32, tag="rcp")
                        nc.vector.reciprocal(rcp[:], tp[:, D:D + 1])
                        o_fin = small_pool.tile([128, D], f32, tag="ofin")
                        nc.vector.tensor_scalar_mul(o_fin[:], tp[:, 0:D], rcp[:])
                        nc.sync.dma_start(
                            o_hbm.ap()[p, ds(lc * LCHUNK + j * 128, 128), :],
                            o_fin[:])

    nc.compile()
    return nc


def _get_program():
    if "nc" not in _PROGRAM_CACHE:
        _PROGRAM_CACHE["nc"] = _build_program()
    return _PROGRAM_CACHE["nc"]


def kernel(queries, keys, values, temp_scale):
    from concourse.bass_utils import run_bass_kernel_spmd

    N, Lq, H, Dh = queries.shape
    assert (N, Lq, H, Dh) == (4, L, 8, D), (N, Lq, H, Dh)

    # [N, L, H, D] -> [N*H, L, D]; core c owns pairs 4c..4c+4.
    def shard(x):
        x = np.ascontiguousarray(
            np.asarray(x, dtype=np.float32).transpose(0, 2, 1, 3)
        ).reshape(N * H, Lq, Dh)
        return [np.ascontiguousarray(x[PAIRS * c:PAIRS * (c + 1)])
                for c in range(N_CORES)]

    qs, ks, vs = shard(queries), shard(keys), shard(values)
    t11 = np.asarray(temp_scale, dtype=np.float32).reshape(1, 1)
    in_maps = [
        {"q": qs[c], "k": ks[c], "v": vs[c], "temp": t11}
        for c in range(N_CORES)
    ]

    nc = _get_program()
    res = run_bass_kernel_spmd(nc, in_maps, core_ids=list(range(N_CORES)))
    if getattr(res, "exec_time_ns", None):
        print(f"HW exec time: {res.exec_time_ns} ns")

    out = np.stack([res.results[c]["o"] for c in range(N_CORES)])  # [8,4,L,D]
    out = out.reshape(N, H, Lq, Dh).transpose(0, 2, 1, 3)          # [N,L,H,D]
    return np.ascontiguousarray(out)


# revision 5
# speedup vs baseline: 1.3152x; 1.3152x over previous
"""Cosine attention kernel for Trainium2, sharded over 8 NeuronCores.

Problem: N=4, L=S=2048, H=8, D=64 fp32.
  q = queries / ||queries||_D ; k = keys / ||keys||_D
  qk = einsum('nlhd,nshd->nlsh', q, k); A = softmax(qk / temp, axis=S)
  out = einsum('nlsh,nshd->nlhd', A, values)

Sharding: the 32 (n, h) pairs are split 4-per-core (data + head parallel).
Each core computes 4 independent 2048x2048 attention problems.

Per-core device algorithm (per pair):
  - load Q, K as [128, 16, 64] tiles (L/S on partitions), V as [128, 16, 65]
    with a ones-column appended (row 64 of the second matmul's output then
    accumulates the softmax denominator).
  - row norms: ssq via DVE square+reduce; rsqrt via ACT ln/exp (keeps all
    ACT traffic in one activation-table set with the softmax Exp);
    1/temp folded into K's row scale.
  - normalize Q,K rows (DVE per-partition scalar), PE-transpose to get
    QnT/KnT [64(D), 2048] — matmul operands with D on partitions.
  - scores transposed: P^T[s_tile, l] = KnT_tile^T @ QnT (fp32r matmuls,
    N=512) into PSUM [128, 1024]; ACT Exp PSUM->SBUF.
  - out^T accumulation: psum2[65, 1024] += V_aug[s]^T @ Pexp[s] over 16
    s-tiles (fp32r); row 64 accumulates sum_s exp = softmax denominator.
  - epilogue: PE-transpose [65,128] blocks back to [128,65], DVE reciprocal
    of the denominator column, per-partition scalar multiply, DMA out.
"""

import sys

if "/opt/trn_rl_repo" not in sys.path:
    sys.path.insert(0, "/opt/trn_rl_repo")

import numpy as np

N_CORES = 8
PAIRS = 4          # (n, h) pairs per core
L = 2048           # query length
S = 2048           # key length
D = 64             # head dim
T = S // 128       # 128-row tiles per pair
LC = 2             # L chunks
LCHUNK = L // LC   # 1024

_PROGRAM_CACHE = {}


def _build_program():
    import concourse.tile as tile
    from concourse import bacc, mybir
    import concourse.bass as bass
    from concourse.bass import ds
    from concourse.masks import make_identity

    f32 = mybir.dt.float32
    f32r = mybir.dt.float32r
    AF = mybir.ActivationFunctionType

    nc = bacc.Bacc("TRN2", target_bir_lowering=False, debug=False,
                   num_devices=N_CORES)
    q_hbm = nc.dram_tensor("q", [PAIRS, L, D], f32, kind="ExternalInput")
    k_hbm = nc.dram_tensor("k", [PAIRS, S, D], f32, kind="ExternalInput")
    v_hbm = nc.dram_tensor("v", [PAIRS, S, D], f32, kind="ExternalInput")
    t_hbm = nc.dram_tensor("temp", [1, 1], f32, kind="ExternalInput")
    o_hbm = nc.dram_tensor("o", [PAIRS, L, D], f32, kind="ExternalOutput")

    with tile.TileContext(nc) as tc:
        with (
            tc.tile_pool(name="const", bufs=1) as cpool,
            tc.tile_pool(name="raw", bufs=1) as raw_pool,
            tc.tile_pool(name="io", bufs=2) as io_pool,
            tc.tile_pool(name="work", bufs=2) as work_pool,
            tc.tile_pool(name="small", bufs=4) as small_pool,
            tc.tile_pool(name="pexp", bufs=3) as pexp_pool,
            tc.tile_pool(name="psum1", bufs=2, space="PSUM") as psum1_pool,
            tc.tile_pool(name="psum2", bufs=1, space="PSUM") as psum2_pool,
            tc.tile_pool(name="psmall", bufs=2, space="PSUM") as psmall_pool,
            tc.tile_pool(name="dram", bufs=1, space="DRAM") as dram_pool,
        ):
            identity = cpool.tile([128, 128], f32)
            make_identity(nc, identity[:])

            # HAM warmup: ~32 dense REGULAR matmuls trip the PE clock gate
            # to K=8/8 (2.4 GHz) while the input DMAs stream in. Transpose-
            # mode PE work does not count as HAM activity, so without this
            # the whole prologue runs at 1.2 GHz.
            identity_r = cpool.tile([128, 128], f32r)
            nc.vector.tensor_copy(identity_r[:], identity[:])
            ps_w = psmall_pool.tile([128, 128], f32, tag="tp", name="ps_w")
            for _ in range(40):
                nc.tensor.matmul(ps_w[:], identity_r[:], identity_r[:])
            warm_sink = cpool.tile([1, 1], f32)
            nc.vector.tensor_copy(warm_sink[:], ps_w[0:1, 0:1])
            warm_dram = dram_pool.tile([1, 1], f32)
            nc.sync.dma_start(warm_dram[:], warm_sink[:])

            # 1/temp broadcast to [128, 1] (bounce through DRAM for the
            # partition-broadcast DMA).
            t_sb = cpool.tile([1, 1], f32)
            nc.sync.dma_start(t_sb[:], t_hbm.ap())
            rt_sb = cpool.tile([1, 1], f32)
            nc.vector.reciprocal(rt_sb[:], t_sb[:])
            rt_dram = dram_pool.tile([1, 1], f32)
            nc.sync.dma_start(rt_dram[:], rt_sb[:])
            rt_b = cpool.tile([128, 1], f32)
            nc.sync.dma_start(rt_b[:], rt_dram[:].to_broadcast([128, 1]))

            # ---- Phase 0: load Q/K for all pairs, compute row-norm scales.
            q_raw, k_raw, rq, rk = {}, {}, {}, {}
            for p in range(PAIRS):
                q_raw[p] = raw_pool.tile([128, T, D], f32, tag=f"qraw{p}", name=f"qraw{p}")
                nc.sync.dma_start(
                    q_raw[p][:],
                    q_hbm.ap()[p].rearrange("(t pp) d -> pp t d", pp=128))
                k_raw[p] = raw_pool.tile([128, T, D], f32, tag=f"kraw{p}", name=f"kraw{p}")
                nc.sync.dma_start(
                    k_raw[p][:],
                    k_hbm.ap()[p].rearrange("(t pp) d -> pp t d", pp=128))

            # All row-norm scales in ONE ssq tile so rsqrt = exp(-0.5 ln) is
            # exactly one Ln + one Exp ACT call — the scheduler cannot
            # interleave them with main-loop Exps, so 2 table loads total.
            ssq_all = cpool.tile([128, 2 * PAIRS, T], f32)
            for p in range(PAIRS):
                for i, srct in ((0, q_raw[p]), (1, k_raw[p])):
                    sq = work_pool.tile([128, T, D], f32, tag="sq")
                    nc.vector.tensor_mul(sq[:], srct[:], srct[:])
                    nc.vector.tensor_reduce(
                        ssq_all[:, 2 * p + i, :], sq[:],
                        axis=mybir.AxisListType.X, op=mybir.AluOpType.add)
            r_all = cpool.tile([128, 2 * PAIRS, T], f32)
            nc.scalar.activation(ssq_all[:], ssq_all[:], AF.Ln)
            nc.scalar.activation(r_all[:], ssq_all[:], AF.Exp, scale=-0.5)
            for p in range(PAIRS):
                rq[p] = r_all[:, 2 * p, :]
                rk[p] = r_all[:, 2 * p + 1, :]
                # fold 1/temp into K's row scale
                nc.vector.tensor_scalar_mul(rk[p], rk[p], rt_b[:])

            # ---- Per-pair pipeline.
            for p in range(PAIRS):
                # V with ones column appended; converted to f32r for mm2.
                v_stage = io_pool.tile([128, T, D + 1], f32, tag="vstage")
                nc.vector.memset(v_stage[:, :, D:D + 1], 1.0)
                nc.sync.dma_start(
                    v_stage[:, :, 0:D],
                    v_hbm.ap()[p].rearrange("(t pp) d -> pp t d", pp=128))
                v_aug = io_pool.tile([128, T, D + 1], f32r, tag="vaug")
                nc.vector.tensor_copy(v_aug[:], v_stage[:])

                # Normalize rows.
                qn = work_pool.tile([128, T, D], f32, tag="qn")
                kn = work_pool.tile([128, T, D], f32, tag="kn")
                for rr, srct, dstt in ((rq[p], q_raw[p], qn), (rk[p], k_raw[p], kn)):
                    r_b = bass.AP(tensor=rr.tensor, offset=rr.offset,
                                  ap=[rr.ap[0], rr.ap[1], [0, D]])
                    nc.vector.tensor_mul(dstt[:], srct[:], r_b)

                # Transpose to [64(D), 2048].
                qnT = work_pool.tile([64, L], f32r, tag="qnT")
                knT = work_pool.tile([64, S], f32r, tag="knT")
                for src, dst in ((qn, qnT), (kn, knT)):
                    for g in range(T // 4):
                        tp = psmall_pool.tile([64, 4, 128], f32, tag="tp")
                        for j in range(4):
                            nc.tensor.transpose(
                                tp[:, j, :], src[:, 4 * g + j, :], identity[:])
                        nc.vector.tensor_copy(dst[:, ds(512 * g, 512)], tp[:])

                # Main loop.
                for lc in range(LC):
                    ps2 = psum2_pool.tile([D + 1, LCHUNK], f32, tag="ps2")
                    for st in range(T):
                        ps1 = psum1_pool.tile([128, LCHUNK], f32, tag="ps1")
                        lhs1 = knT[:, ds(st * 128, 128)]
                        for h in range(LCHUNK // 512):
                            nc.tensor.matmul(
                                ps1[:, ds(h * 512, 512)], lhs1,
                                qnT[:, ds(lc * LCHUNK + h * 512, 512)])
                        pexp = pexp_pool.tile([128, LCHUNK], f32r, tag="pexp")
                        nc.scalar.activation(pexp[:], ps1[:], AF.Exp)
                        lhs2 = v_aug[:, st, :]
                        for h in range(LCHUNK // 512):
                            nc.tensor.matmul(
                                ps2[:, ds(h * 512, 512)], lhs2,
                                pexp[:, ds(h * 512, 512)],
                                start=(st == 0), stop=(st == T - 1))

                    # Epilogue for this L chunk.
                    o_sb = work_pool.tile([D + 1, LCHUNK], f32, tag="osb")
                    nc.vector.tensor_copy(o_sb[:], ps2[:])
                    for j in range(LCHUNK // 128):
                        tp = psmall_pool.tile([128, D + 1], f32, tag="tp")
                        nc.tensor.transpose(
                            tp[:], o_sb[:, ds(j * 128, 128)],
                            identity[0:D + 1, 0:D + 1])
                        rcp = small_pool.tile([128, 1], f32, tag="rcp")
                        nc.vector.reciprocal(rcp[:], tp[:, D:D + 1])
                        o_fin = small_pool.tile([128, D], f32, tag="ofin")
                        nc.vector.tensor_scalar_mul(o_fin[:], tp[:, 0:D], rcp[:])
                        nc.sync.dma_start(
                            o_hbm.ap()[p, ds(lc * LCHUNK + j * 128, 128), :],
                            o_fin[:])

    nc.compile()
    return nc


def _get_program():
    if "nc" not in _PROGRAM_CACHE:
        _PROGRAM_CACHE["nc"] = _build_program()
    return _PROGRAM_CACHE["nc"]


def kernel(queries, keys, values, temp_scale):
    from concourse.bass_utils import run_bass_kernel_spmd

    N, Lq, H, Dh = queries.shape
    assert (N, Lq, H, Dh) == (4, L, 8, D), (N, Lq, H, Dh)

    # [N, L, H, D] -> [N*H, L, D]; core c owns pairs 4c..4c+4.
    def shard(x):
        x = np.ascontiguousarray(
            np.asarray(x, dtype=np.float32).transpose(0, 2, 1, 3)
        ).reshape(N * H, Lq, Dh)
        return [np.ascontiguousarray(x[PAIRS * c:PAIRS * (c + 1)])
                for c in range(N_CORES)]

    qs, ks, vs = shard(queries), shard(keys), shard(values)
    t11 = np.asarray(temp_scale, dtype=np.float32).reshape(1, 1)
    in_maps = [
        {"q": qs[c], "k": ks[c], "v": vs[c], "temp": t11}
        for c in range(N_CORES)
    ]

    nc = _get_program()
    res = run_bass_kernel_spmd(nc, in_maps, core_ids=list(range(N_CORES)))
    if getattr(res, "exec_time_ns", None):
        print(f"HW exec time: {res.exec_time_ns} ns")

    out = np.stack([res.results[c]["o"] for c in range(N_CORES)])  # [8,4,L,D]
    out = out.reshape(N, H, Lq, Dh).transpose(0, 2, 1, 3)          # [N,L,H,D]
    return np.ascontiguousarray(out)
